# revision 14
# baseline (speedup 1.0000x reference)
"""BPR embedding-lookup kernel for 8 TRN2 NeuronCores.

Math (per batch element b):
    out[b] = dot(user_emb[users[b]], item_emb[items[b]])
           + sum_u social_weight[users[b], u] * dot(item_emb[items[b]], user_emb[u])

Key identity: the positive-pair dot is folded into the social term by adding
+1.0 to social_weight[u, u] on the host (delta trick):
    out[b] = sum_u (sw + I)[users[b], u] * dot(uemb[u], bi[b])
           = sum_d biT[d, b] * V[d, b],   V[:, b] = uemb.T @ (sw+I)[users[b], :]

Sharding: sort batch by user index, split into 8 contiguous chunks of 512.
Core m gets the social_weight row range covering its chunk (~1/8 of the
table, +identity on the diagonal slice); output inverse-permuted on host.

Per-core device pipeline (all big traffic = SWDGE transposed gathers that
feed the PE directly):
  - a tiny warm-up gather (memset indices, no DMA dependency) dispatches
    the first Q7 library op ASAP so the ~11us descriptor-library IRAM load
    overlaps the const loads instead of the drain window.
  - (sw+I) rows arrive TRANSPOSED in SBUF via gpsimd dma_gather in 8
    k-slices spread over the 4 SWDGE queues; the queue assignment makes
    slice drains complete staggered in consumption order (SWDGE drain
    bandwidth is shared packet-round-robin across doorbelled queues).
  - biT [64,512] arrives transposed from a per-core deduped item table
    (indices remapped to int16 on host).
  - uembk (user table in k-chunk layout) is ONE contiguous HWDGE load.
  - ~30 dummy matmuls on uembk pre-warm the PE clock (HAM throttle) during
    the library-load window, so real matmuls run at 2.4 GHz immediately.
  - 79 accumulating 512-wide matmuls (two interleaved PSUM chains so
    LDWEIGHTS pipelines under MATMUL, consumed in MM_ORDER = predicted
    drain-completion order) produce V^T in PSUM; tail = 2 DVE multiplies
    by biT (bf16) + 2 accumulating ones-vector matmul reductions over
    d -> out [1, 512], one 2KB store.
"""

import sys

if "/opt/trn_rl_repo" not in sys.path:
    sys.path.insert(0, "/opt/trn_rl_repo")

import numpy as np

NUM_USERS = 10000
NUM_ITEMS = 100000
D = 64
B = 4096
NCORES = 8
BL = B // NCORES          # 512 batch elements per core
UK = 10112                # num_users padded to 79*128 (dma_gather needs 256B elems)
KC = UK // 128            # 79 contraction chunks
S_PAD = 2048              # fixed swp row count (max per-core user span) -> one compile
# sw gather slices in 128-elem chunks over 4 SWDGE queues. Sizes/queues
# chosen by simulating the drain dynamics (SWDGE packet round-robin gives
# each doorbelled queue bandwidth proportional to its head slice's elem
# size; desc-gen serializes per queue, runs parallel across queues).
# MM_ORDER consumes slices in predicted completion order (PSUM
# accumulation is commutative).
SLICES = [2, 10, 20, 20, 17, 4, 2, 4]
QMAP = [1, 3, 0, 1, 3, 0, 1, 3]
BIT_AFTER = 0   # emit biT gather after this slice
BIT_QUEUE = 2
OFFS = [sum(SLICES[:g]) for g in range(len(SLICES))]
MM_ORDER = [0, 1, 2, 3, 5, 6, 4, 7]
N_WARM_MM = 30            # dummy matmuls to hold the PE at 2.4 GHz pre-data

_PROGRAM_CACHE = {}
LAST_RESULTS = None


def _build_program(s_pad: int):
    import ml_dtypes  # noqa: F401

    from concourse import bacc, bass, mybir, tile

    f32 = mybir.dt.float32
    bf16 = mybir.dt.bfloat16
    i16 = mybir.dt.int16
    mult = mybir.AluOpType.mult

    nc = bacc.Bacc(
        "TRN2",
        target_bir_lowering=False,
        debug=False,
        num_devices=NCORES,
        num_swdge_queues=4,
    )
    swp_d = nc.declare_dram_parameter("swp", [s_pad, UK], bf16, isOutput=False)
    # uembk[p, k*D + d] = user_emb_padded[k*128 + p, d]  (contiguous load)
    uembk_d = nc.declare_dram_parameter("uembk", [128, KC * D], bf16, isOutput=False)
    # per-core item embedding rows in batch order, 4 rows packed per gather
    # element (elem 1024B -> full-size SWDGE packets on the shared queue)
    icomp_d = nc.declare_dram_parameter("icomp", [BL // 4, 512], bf16, isOutput=False)
    swidx_d = nc.declare_dram_parameter("swidx", [128, BL // 16], i16, isOutput=False)
    itidx_d = nc.declare_dram_parameter("itidx", [128, BL // 64], i16, isOutput=False)
    ones_d = nc.declare_dram_parameter("ones", [D, 1], bf16, isOutput=False)
    out_d = nc.declare_dram_parameter("out", [1, BL], f32, isOutput=True)

    with tile.TileContext(nc) as tc:
        with (
            tc.tile_pool(name="const", bufs=1) as constp,
            tc.tile_pool(name="swt", bufs=1) as swtp,
            tc.tile_pool(name="small", bufs=4) as smallp,
            tc.tile_pool(name="psum", bufs=2, space="PSUM") as psump,
            tc.tile_pool(name="psum2", bufs=2, space="PSUM") as psum2p,
        ):
            # index / const loads first so SWDGE desc-gen can start ASAP
            swidx_t = constp.tile([128, BL // 16], i16)
            nc.sync.dma_start(out=swidx_t[:], in_=swidx_d[:])
            itidx_t = constp.tile([128, BL // 64], i16)
            nc.sync.dma_start(out=itidx_t[:], in_=itidx_d[:])
            ones_t = constp.tile([D, 1], bf16)
            nc.sync.dma_start(out=ones_t[:], in_=ones_d[:])
            uembk_t = constp.tile([128, KC * D], bf16)
            nc.sync.dma_start(out=uembk_t[:], in_=uembk_d[:])

            # warm-up gather (zero indices via memset, no DMA dependency):
            # dispatches the first library op ASAP so the ~11us Q7 library
            # load overlaps the const loads instead of the drain window.
            warmidx = constp.tile([128, 8], i16)
            nc.gpsimd.memset(warmidx[:], 0)
            warm = smallp.tile([128, 1, 128], bf16, tag="warm")
            nc.gpsimd.dma_gather(
                out_ap=warm[:],
                in_ap=swp_d[:, 0:128],
                idxs_ap=warmidx[:],
                num_idxs=128,
                num_idxs_reg=128,
                elem_size=128,
                elem_step=UK,
                transpose=True,
                queue_num=0,
            )

            # social-weight k-slices, transposed: swt[p, c, b] =
            # swp[row_b, koff + c*128 + p]
            swts = []
            biT = None
            for g, nch in enumerate(SLICES):
                koff = OFFS[g] * 128
                swt = swtp.tile([128, nch, BL], bf16, tag=f"swt{g}")
                nc.gpsimd.dma_gather(
                    out_ap=swt[:],
                    in_ap=swp_d[:, koff : koff + nch * 128],
                    idxs_ap=swidx_t[:],
                    num_idxs=BL,
                    num_idxs_reg=BL,
                    elem_size=nch * 128,
                    elem_step=UK,
                    transpose=True,
                    queue_num=QMAP[g],
                )
                swts.append(swt)
                if g == BIT_AFTER:
                    # biT[d, c, i] = bi[4*i + c, d] (batch-order item rows,
                    # 4 per element; rows 64..127 of each are zero pad)
                    biT = constp.tile([128, 4, BL // 4], bf16)
                    nc.gpsimd.dma_gather(
                        out_ap=biT[:],
                        in_ap=icomp_d[:],
                        idxs_ap=itidx_t[:],
                        num_idxs=BL // 4,
                        num_idxs_reg=BL // 4,
                        elem_size=512,
                        transpose=True,
                        queue_num=BIT_QUEUE,
                    )

            # PE pre-warm: dummy matmuls on uembk keep the HAM clock gate at
            # 2.4 GHz through the Q7 library-load window; results discarded.
            warm_ps = psum2p.tile([D, BL], f32, tag="warm")
            for w in range(N_WARM_MM):
                nc.tensor.matmul(
                    out=warm_ps[:],
                    lhsT=uembk_t[:, 0:D],
                    rhs=uembk_t[:, 0:BL],
                    start=True,
                    stop=True,
                )

            # V^T[d, b] accumulated over all 79 k-chunks (512-wide matmuls).
            # Two interleaved PSUM chains so LDWEIGHTS pipelines under MATMUL.
            vt_ps0 = psump.tile([D, BL], f32, tag="vt0")
            vt_ps1 = psump.tile([D, BL], f32, tag="vt1")
            chains = [vt_ps0, vt_ps1]
            seen = [0, 0]
            per_chain = [sum(1 for k in range(KC) if k % 2 == p) for p in (0, 1)]
            for g in MM_ORDER:
                for c in range(SLICES[g]):
                    k = OFFS[g] + c
                    p = k % 2
                    seen[p] += 1
                    nc.tensor.matmul(
                        out=chains[p][:],
                        lhsT=uembk_t[:, k * D : (k + 1) * D],
                        rhs=swts[g][:, c, :],
                        start=(seen[p] == 1),
                        stop=(seen[p] == per_chain[p]),
                    )

            # tail: out[b] = sum_d biT[d,b] * (vt0 + vt1)[d,b]
            prodA = smallp.tile([D, BL], bf16, tag="prodA")
            nc.vector.tensor_tensor(
                out=prodA[:], in0=vt_ps0[:], in1=biT[:D, :, :], op=mult
            )
            prodB = smallp.tile([D, BL], bf16, tag="prodB")
            nc.vector.tensor_tensor(
                out=prodB[:], in0=vt_ps1[:], in1=biT[:D, :, :], op=mult
            )
            out_ps = psum2p.tile([1, BL], f32, tag="ops")
            nc.tensor.matmul(
                out=out_ps[:], lhsT=ones_t[:], rhs=prodA[:], start=True, stop=False
            )
            nc.tensor.matmul(
                out=out_ps[:], lhsT=ones_t[:], rhs=prodB[:], start=False, stop=True
            )
            out_sb = smallp.tile([1, BL], f32, tag="osb")
            nc.scalar.copy(out=out_sb[:], in_=out_ps[:])
            nc.sync.dma_start(out=out_d[:], in_=out_sb[:])

    nc.finalize()
    return nc


def _wrap16(idx):
    """[BL] int -> [128, BL//16] int16: idx i at (i%16, i//16), replicated x8."""
    n = len(idx)
    blk = np.empty((16, n // 16), np.int16)
    blk[np.arange(n) % 16, np.arange(n) // 16] = idx.astype(np.int16)
    return np.ascontiguousarray(np.tile(blk, (8, 1)))


def kernel(user_emb, item_emb, social_weight, users, items):
    global LAST_RESULTS
    import os

    import ml_dtypes

    from concourse.bass_utils import run_bass_kernel_spmd

    bf = ml_dtypes.bfloat16
    user_emb = np.ascontiguousarray(np.asarray(user_emb, dtype=np.float32))
    item_emb = np.ascontiguousarray(np.asarray(item_emb, dtype=np.float32))
    social_weight = np.ascontiguousarray(np.asarray(social_weight, dtype=np.float32))
    users = np.asarray(users).astype(np.int64)
    items = np.asarray(items).astype(np.int64)

    order = np.argsort(users, kind="stable")
    users_s = users[order]
    items_s = items[order]

    los, spans = [], []
    for m in range(NCORES):
        seg = users_s[m * BL : (m + 1) * BL]
        lo = int(seg[0])
        hi = int(seg[-1]) + 1
        los.append(lo)
        spans.append(hi - lo)
    s_pad = max(S_PAD, 256 * ((max(spans) + 255) // 256))

    if s_pad not in _PROGRAM_CACHE:
        _PROGRAM_CACHE[s_pad] = _build_program(s_pad)
    nc = _PROGRAM_CACHE[s_pad]

    uembk_pad = np.zeros((UK, D), np.float32)
    uembk_pad[:NUM_USERS] = user_emb
    # [128, KC*D] with uembk[p, k*D+d] = uemb_pad[k*128+p, d]
    uembk = np.ascontiguousarray(
        uembk_pad.reshape(KC, 128, D).transpose(1, 0, 2).reshape(128, KC * D)
    ).astype(bf)
    ones = np.ones((D, 1), bf)

    in_maps = []
    for m in range(NCORES):
        seg_ug = users_s[m * BL : (m + 1) * BL]
        seg_u = (seg_ug - los[m]).astype(np.int64)
        seg_i = items_s[m * BL : (m + 1) * BL]
        # social_weight rows for this core's user span, +1.0 on the diagonal
        # (delta trick: folds the user.item positive dot into the matmul)
        sw_rows = social_weight[los[m] : los[m] + spans[m]].copy()
        jj = np.arange(spans[m])
        sw_rows[jj, los[m] + jj] += 1.0
        swp = np.zeros((s_pad, UK), bf)
        swp[: spans[m], :NUM_USERS] = sw_rows.astype(bf)
        # item rows packed 4 per gather element; row 4*i + c holds batch
        # c*128 + i so the transposed gather's (c, i) free layout is
        # batch-major with no on-device rearrange
        r = np.arange(BL)
        perm = (r % 4) * 128 + r // 4
        icomp = np.zeros((BL, 128), bf)
        icomp[:, :D] = item_emb[seg_i[perm]].astype(bf)
        icomp = icomp.reshape(BL // 4, 512)
        inv = np.arange(BL // 4)
        in_maps.append(
            {
                "swp": swp,
                "uembk": uembk,
                "icomp": icomp,
                "swidx": _wrap16(seg_u),
                "itidx": _wrap16(inv),
                "ones": ones,
            }
        )

    trace = bool(os.environ.get("CC_KERNEL_TRACE"))
    tmpdir = os.environ.get("CC_TRACE_DIR") or None
    res = run_bass_kernel_spmd(
        nc, in_maps, list(range(NCORES)), trace=trace, tmpdir=tmpdir
    )
    LAST_RESULTS = res

    out_sorted = np.empty(B, np.float32)
    for m in range(NCORES):
        o = np.asarray(res.results[m]["out"])  # [1, BL]
        out_sorted[m * BL : (m + 1) * BL] = o.reshape(-1)

    final = np.empty(B, np.float32)
    final[order] = out_sorted
    return final


# revision 15
# speedup vs baseline: 1.0178x; 1.0178x over previous
"""BPR embedding-lookup kernel for 8 TRN2 NeuronCores.

Math (per batch element b):
    out[b] = dot(user_emb[users[b]], item_emb[items[b]])
           + sum_u social_weight[users[b], u] * dot(item_emb[items[b]], user_emb[u])

Key identity: the positive-pair dot is folded into the social term by adding
+1.0 to social_weight[u, u] on the host (delta trick):
    out[b] = sum_u (sw + I)[users[b], u] * dot(uemb[u], bi[b])
           = sum_d biT[d, b] * V[d, b],   V[:, b] = uemb.T @ (sw+I)[users[b], :]

Sharding: sort batch by user index, split into 8 contiguous chunks of 512.
Core m gets the social_weight row range covering its chunk (~1/8 of the
table, +identity on the diagonal slice); output inverse-permuted on host.

Per-core device pipeline (all big traffic = SWDGE transposed gathers that
feed the PE directly):
  - a tiny warm-up gather (memset indices, no DMA dependency) dispatches
    the first Q7 library op ASAP so the ~11us descriptor-library IRAM load
    overlaps the const loads instead of the drain window.
  - (sw+I) rows arrive TRANSPOSED in SBUF via gpsimd dma_gather in 8
    k-slices spread over the 4 SWDGE queues; the queue assignment makes
    slice drains complete staggered in consumption order (SWDGE drain
    bandwidth is shared packet-round-robin across doorbelled queues).
  - biT [64,512] arrives transposed from a per-core deduped item table
    (indices remapped to int16 on host).
  - uembk (user table in k-chunk layout) is ONE contiguous HWDGE load.
  - ~30 dummy matmuls on uembk pre-warm the PE clock (HAM throttle) during
    the library-load window, so real matmuls run at 2.4 GHz immediately.
  - 79 accumulating 512-wide matmuls (two interleaved PSUM chains so
    LDWEIGHTS pipelines under MATMUL, consumed in MM_ORDER = predicted
    drain-completion order) produce V^T in PSUM; tail = 2 DVE multiplies
    by biT (bf16) + 2 accumulating ones-vector matmul reductions over
    d -> out [1, 512], one 2KB store.
"""

import sys

if "/opt/trn_rl_repo" not in sys.path:
    sys.path.insert(0, "/opt/trn_rl_repo")

import numpy as np

NUM_USERS = 10000
NUM_ITEMS = 100000
D = 64
B = 4096
NCORES = 8
BL = B // NCORES          # 512 batch elements per core
UK = 10112                # num_users padded to 79*128 (dma_gather needs 256B elems)
KC = UK // 128            # 79 contraction chunks
S_PAD = 2048              # fixed swp row count (max per-core user span) -> one compile
# sw gather slices in 128-elem chunks over 4 SWDGE queues. Sizes/queues
# chosen by simulating the drain dynamics (SWDGE packet round-robin gives
# each doorbelled queue bandwidth proportional to its head slice's elem
# size; desc-gen serializes per queue, runs parallel across queues).
# MM_ORDER consumes slices in predicted completion order (PSUM
# accumulation is commutative).
SLICES = [2, 10, 13, 13, 13, 13, 13, 2]
QMAP = [1, 3, 0, 1, 2, 3, 0, 1]
BIT_AFTER = 0   # emit biT gather after this slice
BIT_QUEUE = 2
OFFS = [sum(SLICES[:g]) for g in range(len(SLICES))]
MM_ORDER = [0, 1, 2, 4, 3, 5, 6, 7]
N_WARM_MM = 30            # dummy matmuls to hold the PE at 2.4 GHz pre-data

_PROGRAM_CACHE = {}
LAST_RESULTS = None


def _build_program(s_pad: int):
    import ml_dtypes  # noqa: F401

    from concourse import bacc, bass, mybir, tile

    f32 = mybir.dt.float32
    bf16 = mybir.dt.bfloat16
    i16 = mybir.dt.int16
    mult = mybir.AluOpType.mult

    nc = bacc.Bacc(
        "TRN2",
        target_bir_lowering=False,
        debug=False,
        num_devices=NCORES,
        num_swdge_queues=4,
    )
    swp_d = nc.declare_dram_parameter("swp", [s_pad, UK], bf16, isOutput=False)
    # uembk[p, k*D + d] = user_emb_padded[k*128 + p, d]  (contiguous load)
    uembk_d = nc.declare_dram_parameter("uembk", [128, KC * D], bf16, isOutput=False)
    # per-core item embedding rows in batch order, 4 rows packed per gather
    # element (elem 1024B -> full-size SWDGE packets on the shared queue)
    icomp_d = nc.declare_dram_parameter("icomp", [BL // 4, 512], bf16, isOutput=False)
    swidx_d = nc.declare_dram_parameter("swidx", [128, BL // 16], i16, isOutput=False)
    itidx_d = nc.declare_dram_parameter("itidx", [128, BL // 64], i16, isOutput=False)
    ones_d = nc.declare_dram_parameter("ones", [D, 1], bf16, isOutput=False)
    out_d = nc.declare_dram_parameter("out", [1, BL], f32, isOutput=True)

    with tile.TileContext(nc) as tc:
        with (
            tc.tile_pool(name="const", bufs=1) as constp,
            tc.tile_pool(name="swt", bufs=1) as swtp,
            tc.tile_pool(name="small", bufs=4) as smallp,
            tc.tile_pool(name="psum", bufs=2, space="PSUM") as psump,
            tc.tile_pool(name="psum2", bufs=2, space="PSUM") as psum2p,
        ):
            # index / const loads first so SWDGE desc-gen can start ASAP
            swidx_t = constp.tile([128, BL // 16], i16)
            nc.sync.dma_start(out=swidx_t[:], in_=swidx_d[:])
            itidx_t = constp.tile([128, BL // 64], i16)
            nc.sync.dma_start(out=itidx_t[:], in_=itidx_d[:])
            ones_t = constp.tile([D, 1], bf16)
            nc.sync.dma_start(out=ones_t[:], in_=ones_d[:])
            uembk_t = constp.tile([128, KC * D], bf16)
            nc.sync.dma_start(out=uembk_t[:], in_=uembk_d[:])

            # warm-up gather (zero indices via memset, no DMA dependency):
            # dispatches the first library op ASAP so the ~11us Q7 library
            # load overlaps the const loads instead of the drain window.
            warmidx = constp.tile([128, 8], i16)
            nc.gpsimd.memset(warmidx[:], 0)
            warm = smallp.tile([128, 1, 128], bf16, tag="warm")
            nc.gpsimd.dma_gather(
                out_ap=warm[:],
                in_ap=swp_d[:, 0:128],
                idxs_ap=warmidx[:],
                num_idxs=128,
                num_idxs_reg=128,
                elem_size=128,
                elem_step=UK,
                transpose=True,
                queue_num=0,
            )

            # social-weight k-slices, transposed: swt[p, c, b] =
            # swp[row_b, koff + c*128 + p]
            swts = []
            biT = None
            for g, nch in enumerate(SLICES):
                koff = OFFS[g] * 128
                swt = swtp.tile([128, nch, BL], bf16, tag=f"swt{g}")
                nc.gpsimd.dma_gather(
                    out_ap=swt[:],
                    in_ap=swp_d[:, koff : koff + nch * 128],
                    idxs_ap=swidx_t[:],
                    num_idxs=BL,
                    num_idxs_reg=BL,
                    elem_size=nch * 128,
                    elem_step=UK,
                    transpose=True,
                    queue_num=QMAP[g],
                )
                swts.append(swt)
                if g == BIT_AFTER:
                    # biT[d, c, i] = bi[4*i + c, d] (batch-order item rows,
                    # 4 per element; rows 64..127 of each are zero pad)
                    biT = constp.tile([128, 4, BL // 4], bf16)
                    nc.gpsimd.dma_gather(
                        out_ap=biT[:],
                        in_ap=icomp_d[:],
                        idxs_ap=itidx_t[:],
                        num_idxs=BL // 4,
                        num_idxs_reg=BL // 4,
                        elem_size=512,
                        transpose=True,
                        queue_num=BIT_QUEUE,
                    )

            # PE pre-warm: dummy matmuls on uembk keep the HAM clock gate at
            # 2.4 GHz through the Q7 library-load window; results discarded.
            warm_ps = psum2p.tile([D, BL], f32, tag="warm")
            for w in range(N_WARM_MM):
                nc.tensor.matmul(
                    out=warm_ps[:],
                    lhsT=uembk_t[:, 0:D],
                    rhs=uembk_t[:, 0:BL],
                    start=True,
                    stop=True,
                )

            # V^T[d, b] accumulated over all 79 k-chunks (512-wide matmuls).
            # Two interleaved PSUM chains so LDWEIGHTS pipelines under MATMUL.
            vt_ps0 = psump.tile([D, BL], f32, tag="vt0")
            vt_ps1 = psump.tile([D, BL], f32, tag="vt1")
            chains = [vt_ps0, vt_ps1]
            seen = [0, 0]
            per_chain = [sum(1 for k in range(KC) if k % 2 == p) for p in (0, 1)]
            for g in MM_ORDER:
                for c in range(SLICES[g]):
                    k = OFFS[g] + c
                    p = k % 2
                    seen[p] += 1
                    nc.tensor.matmul(
                        out=chains[p][:],
                        lhsT=uembk_t[:, k * D : (k + 1) * D],
                        rhs=swts[g][:, c, :],
                        start=(seen[p] == 1),
                        stop=(seen[p] == per_chain[p]),
                    )

            # tail: out[b] = sum_d biT[d,b] * (vt0 + vt1)[d,b]
            prodA = smallp.tile([D, BL], bf16, tag="prodA")
            nc.vector.tensor_tensor(
                out=prodA[:], in0=vt_ps0[:], in1=biT[:D, :, :], op=mult
            )
            prodB = smallp.tile([D, BL], bf16, tag="prodB")
            nc.vector.tensor_tensor(
                out=prodB[:], in0=vt_ps1[:], in1=biT[:D, :, :], op=mult
            )
            out_ps = psum2p.tile([1, BL], f32, tag="ops")
            nc.tensor.matmul(
                out=out_ps[:], lhsT=ones_t[:], rhs=prodA[:], start=True, stop=False
            )
            nc.tensor.matmul(
                out=out_ps[:], lhsT=ones_t[:], rhs=prodB[:], start=False, stop=True
            )
            out_sb = smallp.tile([1, BL], f32, tag="osb")
            nc.scalar.copy(out=out_sb[:], in_=out_ps[:])
            nc.sync.dma_start(out=out_d[:], in_=out_sb[:])

    nc.finalize()
    return nc


def _wrap16(idx):
    """[BL] int -> [128, BL//16] int16: idx i at (i%16, i//16), replicated x8."""
    n = len(idx)
    blk = np.empty((16, n // 16), np.int16)
    blk[np.arange(n) % 16, np.arange(n) // 16] = idx.astype(np.int16)
    return np.ascontiguousarray(np.tile(blk, (8, 1)))


def kernel(user_emb, item_emb, social_weight, users, items):
    global LAST_RESULTS
    import os

    import ml_dtypes

    from concourse.bass_utils import run_bass_kernel_spmd

    bf = ml_dtypes.bfloat16
    user_emb = np.ascontiguousarray(np.asarray(user_emb, dtype=np.float32))
    item_emb = np.ascontiguousarray(np.asarray(item_emb, dtype=np.float32))
    social_weight = np.ascontiguousarray(np.asarray(social_weight, dtype=np.float32))
    users = np.asarray(users).astype(np.int64)
    items = np.asarray(items).astype(np.int64)

    order = np.argsort(users, kind="stable")
    users_s = users[order]
    items_s = items[order]

    los, spans = [], []
    for m in range(NCORES):
        seg = users_s[m * BL : (m + 1) * BL]
        lo = int(seg[0])
        hi = int(seg[-1]) + 1
        los.append(lo)
        spans.append(hi - lo)
    s_pad = max(S_PAD, 256 * ((max(spans) + 255) // 256))

    if s_pad not in _PROGRAM_CACHE:
        _PROGRAM_CACHE[s_pad] = _build_program(s_pad)
    nc = _PROGRAM_CACHE[s_pad]

    uembk_pad = np.zeros((UK, D), np.float32)
    uembk_pad[:NUM_USERS] = user_emb
    # [128, KC*D] with uembk[p, k*D+d] = uemb_pad[k*128+p, d]
    uembk = np.ascontiguousarray(
        uembk_pad.reshape(KC, 128, D).transpose(1, 0, 2).reshape(128, KC * D)
    ).astype(bf)
    ones = np.ones((D, 1), bf)

    in_maps = []
    for m in range(NCORES):
        seg_ug = users_s[m * BL : (m + 1) * BL]
        seg_u = (seg_ug - los[m]).astype(np.int64)
        seg_i = items_s[m * BL : (m + 1) * BL]
        # social_weight rows for this core's user span, +1.0 on the diagonal
        # (delta trick: folds the user.item positive dot into the matmul)
        sw_rows = social_weight[los[m] : los[m] + spans[m]].copy()
        jj = np.arange(spans[m])
        sw_rows[jj, los[m] + jj] += 1.0
        swp = np.zeros((s_pad, UK), bf)
        swp[: spans[m], :NUM_USERS] = sw_rows.astype(bf)
        # item rows packed 4 per gather element; row 4*i + c holds batch
        # c*128 + i so the transposed gather's (c, i) free layout is
        # batch-major with no on-device rearrange
        r = np.arange(BL)
        perm = (r % 4) * 128 + r // 4
        icomp = np.zeros((BL, 128), bf)
        icomp[:, :D] = item_emb[seg_i[perm]].astype(bf)
        icomp = icomp.reshape(BL // 4, 512)
        inv = np.arange(BL // 4)
        in_maps.append(
            {
                "swp": swp,
                "uembk": uembk,
                "icomp": icomp,
                "swidx": _wrap16(seg_u),
                "itidx": _wrap16(inv),
                "ones": ones,
            }
        )

    trace = bool(os.environ.get("CC_KERNEL_TRACE"))
    tmpdir = os.environ.get("CC_TRACE_DIR") or None
    res = run_bass_kernel_spmd(
        nc, in_maps, list(range(NCORES)), trace=trace, tmpdir=tmpdir
    )
    LAST_RESULTS = res

    out_sorted = np.empty(B, np.float32)
    for m in range(NCORES):
        o = np.asarray(res.results[m]["out"])  # [1, BL]
        out_sorted[m * BL : (m + 1) * BL] = o.reshape(-1)

    final = np.empty(B, np.float32)
    final[order] = out_sorted
    return final


# revision 17
# speedup vs baseline: 1.0259x; 1.0080x over previous
"""BPR embedding-lookup kernel for 8 TRN2 NeuronCores.

Math (per batch element b):
    out[b] = dot(user_emb[users[b]], item_emb[items[b]])
           + sum_u social_weight[users[b], u] * dot(item_emb[items[b]], user_emb[u])

Key identity: the positive-pair dot is folded into the social term by adding
+1.0 to social_weight[u, u] on the host (delta trick):
    out[b] = sum_u (sw + I)[users[b], u] * dot(uemb[u], bi[b])
           = sum_d biT[d, b] * V[d, b],   V[:, b] = uemb.T @ (sw+I)[users[b], :]

Sharding: sort batch by user index, split into 8 contiguous chunks of 512.
Core m gets the social_weight row range covering its chunk (~1/8 of the
table, +identity on the diagonal slice); output inverse-permuted on host.

Per-core device pipeline (all big traffic = SWDGE transposed gathers that
feed the PE directly):
  - a tiny warm-up gather (memset indices, no DMA dependency) dispatches
    the first Q7 library op ASAP so the ~11us descriptor-library IRAM load
    overlaps the const loads instead of the drain window.
  - (sw+I) rows arrive TRANSPOSED in SBUF via gpsimd dma_gather in 8
    k-slices spread over the 4 SWDGE queues; the queue assignment makes
    slice drains complete staggered in consumption order (SWDGE drain
    bandwidth is shared packet-round-robin across doorbelled queues).
  - biT [64,512] arrives transposed from a per-core deduped item table
    (indices remapped to int16 on host).
  - uembk (user table in k-chunk layout) is ONE contiguous HWDGE load.
  - ~30 dummy matmuls on uembk pre-warm the PE clock (HAM throttle) during
    the library-load window, so real matmuls run at 2.4 GHz immediately.
  - 79 accumulating 512-wide matmuls (two interleaved PSUM chains so
    LDWEIGHTS pipelines under MATMUL, consumed in MM_ORDER = predicted
    drain-completion order) produce V^T in PSUM; tail = 2 DVE multiplies
    by biT (bf16) + 2 accumulating ones-vector matmul reductions over
    d -> out [1, 512], one 2KB store.
"""

import sys

if "/opt/trn_rl_repo" not in sys.path:
    sys.path.insert(0, "/opt/trn_rl_repo")

import numpy as np

NUM_USERS = 10000
NUM_ITEMS = 100000
D = 64
B = 4096
NCORES = 8
BL = B // NCORES          # 512 batch elements per core
UK = 10112                # num_users padded to 79*128 (dma_gather needs 256B elems)
KC = UK // 128            # 79 contraction chunks
S_PAD = 2048              # fixed swp row count (max per-core user span) -> one compile
# sw gather slices in 128-elem chunks over 4 SWDGE queues. Sizes/queues
# chosen by simulating the drain dynamics (SWDGE packet round-robin gives
# each doorbelled queue bandwidth proportional to its head slice's elem
# size; desc-gen serializes per queue, runs parallel across queues).
# MM_ORDER consumes slices in predicted completion order (PSUM
# accumulation is commutative).
SLICES = [2, 10, 13, 13, 13, 13, 13, 2]
QMAP = [1, 3, 0, 1, 2, 3, 0, 1]
BIT_AFTER = 0   # emit biT gather after this slice
BIT_QUEUE = 2
OFFS = [sum(SLICES[:g]) for g in range(len(SLICES))]
MM_ORDER = [0, 1, 2, 4, 3, 5, 6, 7]
N_WARM_MM = 30            # dummy matmuls to hold the PE at 2.4 GHz pre-data

_PROGRAM_CACHE = {}
LAST_RESULTS = None


def _build_program(s_pad: int):
    import ml_dtypes  # noqa: F401

    from concourse import bacc, bass, mybir, tile

    f32 = mybir.dt.float32
    bf16 = mybir.dt.bfloat16
    i16 = mybir.dt.int16
    mult = mybir.AluOpType.mult

    nc = bacc.Bacc(
        "TRN2",
        target_bir_lowering=False,
        debug=False,
        num_devices=NCORES,
        num_swdge_queues=4,
    )
    swp_d = nc.declare_dram_parameter("swp", [s_pad, UK], bf16, isOutput=False)
    # uembk[p, k*D + d] = user_emb_padded[k*128 + p, d]  (contiguous load)
    uembk_d = nc.declare_dram_parameter("uembk", [128, KC * D], bf16, isOutput=False)
    # per-core item embedding rows in batch order, 4 rows packed per gather
    # element (elem 1024B -> full-size SWDGE packets on the shared queue)
    icomp_d = nc.declare_dram_parameter("icomp", [BL // 4, 512], bf16, isOutput=False)
    swidx_d = nc.declare_dram_parameter("swidx", [128, BL // 16], i16, isOutput=False)
    itidx_d = nc.declare_dram_parameter("itidx", [128, BL // 64], i16, isOutput=False)
    ones_d = nc.declare_dram_parameter("ones", [D, 1], bf16, isOutput=False)
    out_d = nc.declare_dram_parameter("out", [1, BL], f32, isOutput=True)

    with tile.TileContext(nc) as tc:
        with (
            tc.tile_pool(name="const", bufs=1) as constp,
            tc.tile_pool(name="swt", bufs=1) as swtp,
            tc.tile_pool(name="small", bufs=4) as smallp,
            tc.tile_pool(name="psum", bufs=2, space="PSUM") as psump,
            tc.tile_pool(name="psum2", bufs=2, space="PSUM") as psum2p,
        ):
            # index / const loads first so SWDGE desc-gen can start ASAP
            swidx_t = constp.tile([128, BL // 16], i16)
            nc.sync.dma_start(out=swidx_t[:], in_=swidx_d[:])
            itidx_t = constp.tile([128, BL // 64], i16)
            nc.sync.dma_start(out=itidx_t[:], in_=itidx_d[:])
            ones_t = constp.tile([D, 1], bf16)
            nc.sync.dma_start(out=ones_t[:], in_=ones_d[:])
            uembk_t = constp.tile([128, KC * D], bf16)
            nc.sync.dma_start(out=uembk_t[:], in_=uembk_d[:])

            # warm-up gather (zero indices via memset, no DMA dependency):
            # dispatches the first library op ASAP so the ~11us Q7 library
            # load overlaps the const loads instead of the drain window.
            warmidx = constp.tile([128, 8], i16)
            nc.gpsimd.memset(warmidx[:], 0)
            warm = smallp.tile([128, 1, 128], bf16, tag="warm")
            nc.gpsimd.dma_gather(
                out_ap=warm[:],
                in_ap=swp_d[:, 0:128],
                idxs_ap=warmidx[:],
                num_idxs=128,
                num_idxs_reg=128,
                elem_size=128,
                elem_step=UK,
                transpose=True,
                queue_num=0,
            )

            # social-weight k-slices, transposed: swt[p, c, b] =
            # swp[row_b, koff + c*128 + p]
            swts = []
            biT = None
            for g, nch in enumerate(SLICES):
                koff = OFFS[g] * 128
                swt = swtp.tile([128, nch, BL], bf16, tag=f"swt{g}")
                nc.gpsimd.dma_gather(
                    out_ap=swt[:],
                    in_ap=swp_d[:, koff : koff + nch * 128],
                    idxs_ap=swidx_t[:],
                    num_idxs=BL,
                    num_idxs_reg=BL,
                    elem_size=nch * 128,
                    elem_step=UK,
                    transpose=True,
                    queue_num=QMAP[g],
                )
                swts.append(swt)
                if g == BIT_AFTER:
                    # biT[d, c, i] = bi[4*i + c, d] (batch-order item rows,
                    # 4 per element; rows 64..127 of each are zero pad)
                    biT = constp.tile([128, 4, BL // 4], bf16)
                    nc.gpsimd.dma_gather(
                        out_ap=biT[:],
                        in_ap=icomp_d[:],
                        idxs_ap=itidx_t[:],
                        num_idxs=BL // 4,
                        num_idxs_reg=BL // 4,
                        elem_size=512,
                        transpose=True,
                        queue_num=BIT_QUEUE,
                    )

            # PE pre-warm: dummy matmuls on uembk keep the HAM clock gate at
            # 2.4 GHz through the Q7 library-load window; results discarded.
            warm_ps = psum2p.tile([D, BL], f32, tag="warm")
            for w in range(N_WARM_MM):
                nc.tensor.matmul(
                    out=warm_ps[:],
                    lhsT=uembk_t[:, 0:D],
                    rhs=uembk_t[:, 0:BL],
                    start=True,
                    stop=True,
                )

            # V^T[d, b] accumulated over all 79 k-chunks (512-wide matmuls).
            # Two interleaved PSUM chains so LDWEIGHTS pipelines under MATMUL.
            vt_ps0 = psump.tile([D, BL], f32, tag="vt0")
            vt_ps1 = psump.tile([D, BL], f32, tag="vt1")
            chains = [vt_ps0, vt_ps1]
            seen = [0, 0]
            per_chain = [sum(1 for k in range(KC) if k % 2 == p) for p in (0, 1)]
            for g in MM_ORDER:
                for c in range(SLICES[g]):
                    k = OFFS[g] + c
                    p = k % 2
                    seen[p] += 1
                    nc.tensor.matmul(
                        out=chains[p][:],
                        lhsT=uembk_t[:, k * D : (k + 1) * D],
                        rhs=swts[g][:, c, :],
                        start=(seen[p] == 1),
                        stop=(seen[p] == per_chain[p]),
                    )

            # tail: out[b] = sum_d biT[d,b] * (vt0 + vt1)[d,b]
            prodA = smallp.tile([D, BL], bf16, tag="prodA")
            nc.vector.tensor_tensor(
                out=prodA[:], in0=vt_ps0[:], in1=biT[:D, :, :], op=mult
            )
            prodB = smallp.tile([D, BL], bf16, tag="prodB")
            nc.vector.tensor_tensor(
                out=prodB[:], in0=vt_ps1[:], in1=biT[:D, :, :], op=mult
            )
            out_ps = psum2p.tile([1, BL], f32, tag="ops")
            nc.tensor.matmul(
                out=out_ps[:], lhsT=ones_t[:], rhs=prodA[:], start=True, stop=False
            )
            nc.tensor.matmul(
                out=out_ps[:], lhsT=ones_t[:], rhs=prodB[:], start=False, stop=True
            )
            out_sb = smallp.tile([1, BL], f32, tag="osb")
            nc.scalar.copy(out=out_sb[:], in_=out_ps[:])
            nc.sync.dma_start(out=out_d[:], in_=out_sb[:])

    nc.finalize()
    return nc


def _wrap16(idx):
    """[BL] int -> [128, BL//16] int16: idx i at (i%16, i//16), replicated x8."""
    n = len(idx)
    blk = np.empty((16, n // 16), np.int16)
    blk[np.arange(n) % 16, np.arange(n) // 16] = idx.astype(np.int16)
    return np.ascontiguousarray(np.tile(blk, (8, 1)))


def kernel(user_emb, item_emb, social_weight, users, items):
    global LAST_RESULTS
    import os

    import ml_dtypes

    from concourse.bass_utils import run_bass_kernel_spmd

    bf = ml_dtypes.bfloat16
    user_emb = np.ascontiguousarray(np.asarray(user_emb, dtype=np.float32))
    item_emb = np.ascontiguousarray(np.asarray(item_emb, dtype=np.float32))
    social_weight = np.ascontiguousarray(np.asarray(social_weight, dtype=np.float32))
    users = np.asarray(users).astype(np.int64)
    items = np.asarray(items).astype(np.int64)

    order = np.argsort(users, kind="stable")
    users_s = users[order]
    items_s = items[order]

    los, spans = [], []
    for m in range(NCORES):
        seg = users_s[m * BL : (m + 1) * BL]
        lo = int(seg[0])
        hi = int(seg[-1]) + 1
        los.append(lo)
        spans.append(hi - lo)
    s_pad = max(S_PAD, 256 * ((max(spans) + 255) // 256))

    if s_pad not in _PROGRAM_CACHE:
        _PROGRAM_CACHE[s_pad] = _build_program(s_pad)
    nc = _PROGRAM_CACHE[s_pad]

    uembk_pad = np.zeros((UK, D), np.float32)
    uembk_pad[:NUM_USERS] = user_emb
    # [128, KC*D] with uembk[p, k*D+d] = uemb_pad[k*128+p, d]
    uembk = np.ascontiguousarray(
        uembk_pad.reshape(KC, 128, D).transpose(1, 0, 2).reshape(128, KC * D)
    ).astype(bf)
    ones = np.ones((D, 1), bf)

    in_maps = []
    for m in range(NCORES):
        seg_ug = users_s[m * BL : (m + 1) * BL]
        seg_u = (seg_ug - los[m]).astype(np.int64)
        seg_i = items_s[m * BL : (m + 1) * BL]
        # social_weight rows for this core's user span, +1.0 on the diagonal
        # (delta trick: folds the user.item positive dot into the matmul)
        sw_rows = social_weight[los[m] : los[m] + spans[m]].copy()
        jj = np.arange(spans[m])
        sw_rows[jj, los[m] + jj] += 1.0
        swp = np.zeros((s_pad, UK), bf)
        swp[: spans[m], :NUM_USERS] = sw_rows.astype(bf)
        # item rows packed 4 per gather element; row 4*i + c holds batch
        # c*128 + i so the transposed gather's (c, i) free layout is
        # batch-major with no on-device rearrange
        r = np.arange(BL)
        perm = (r % 4) * 128 + r // 4
        icomp = np.zeros((BL, 128), bf)
        icomp[:, :D] = item_emb[seg_i[perm]].astype(bf)
        icomp = icomp.reshape(BL // 4, 512)
        inv = np.arange(BL // 4)
        in_maps.append(
            {
                "swp": swp,
                "uembk": uembk,
                "icomp": icomp,
                "swidx": _wrap16(seg_u),
                "itidx": _wrap16(inv),
                "ones": ones,
            }
        )

    trace = bool(os.environ.get("CC_KERNEL_TRACE"))
    tmpdir = os.environ.get("CC_TRACE_DIR") or None
    res = run_bass_kernel_spmd(
        nc, in_maps, list(range(NCORES)), trace=trace, tmpdir=tmpdir
    )
    LAST_RESULTS = res

    out_sorted = np.empty(B, np.float32)
    for m in range(NCORES):
        o = np.asarray(res.results[m]["out"])  # [1, BL]
        out_sorted[m * BL : (m + 1) * BL] = o.reshape(-1)

    final = np.empty(B, np.float32)
    final[order] = out_sorted
    return final
